# revision 18
# baseline (speedup 1.0000x reference)
"""Trainium2 Bass kernel for nn_DirectionalConv (moe_routing).

Math: out = (1/8) * sum_k conv3x3(x * [octant(sobel(x)) == k], W[k]) + mean_k b[k]

Implementation notes:
- Data-parallel over batch B=8 across 8 NeuronCores (one image per core).
- Octant selection is rewritten in a +-1 "monomial" basis over the three sign
  bits (sign(gy), sign(gx), sign(|gy|-|gx|)):
      sum_k conv(x*mask_k, W[k]) = sum_{S in 2^3} conv(x*chi_S, W'_S)
  where chi_S = product of the selected signs (computed with pure bitwise
  XOR of sign bits - exact) and W'_S = (1/64) sum_k chi_S(k) W[k] is
  precomputed on the host.  This gives 8 dense 3x3 convs, evaluated as
  9 shifted matmuls each, accumulating in PSUM.
- Per-core image (64,256,256) is split into top/bottom halves across the
  SBUF partition dimension: partition p = (half<<6)|channel.  Conv matmuls
  are K=64 and use 4-way PE tile packing (2 row-groups x 2 col-groups) to
  fill the 128x128 array.
- Sobel gradients are computed in fp32 (fp16 anywhere in the gradient/bin
  path measurably fails the 2e-2 gate: even bins from fp16(x) give 2.2e-2).
- Scheduling (vs the 591us baseline): PE is at its fp16 streaming floor
  (~491us busy), so the wins are overlap-only:
    * ACT function table warmed by a dummy activation at t=0.
    * x chunk DMA split across the two HW DMA-gen queues (SP + Activation).
    * gy-path (ut,b2,gy32) computed before the gx-path so the first
      monomial sign source is ready ~8us earlier per chunk.
    * full software pipelining: chunk i+1's monomial production (DVE ~29.3
      us/chunk) runs entirely during chunk i's matmuls (PE 32us/chunk), so
      the ACT casts are never queued behind end-of-chunk PSUM evacs.
    * matmuls m-MAJOR (all 72 tap-groups of a monomial before the next) for
      2x fill runway; monomials consumed in availability order
      (0,4,2,6,1,5,3,7).
"""

import numpy as np

import concourse.bacc as bacc
import concourse.bass as bass
import concourse.mybir as mybir
from concourse import bass_utils
from concourse.tile import TileContext

F32 = mybir.dt.float32
F16 = mybir.dt.float16
U32 = mybir.dt.uint32
ALU = mybir.AluOpType
ACTF = mybir.ActivationFunctionType

B, C, H, W_, K, O = 8, 64, 256, 256, 8, 64
HH = H // 2          # rows per half
R = 8                # output rows per half per chunk
NCHUNK = HH // R     # 16
WP = W_ + 2          # padded width 258
SIGN16 = 0x80008000  # sign bits of two packed fp16 lanes
MORDER = (0, 4, 2, 6, 1, 5, 3, 7)  # monomial availability order


def _build_nc():
    nc = bacc.Bacc("TRN2", target_bir_lowering=False, debug=False)

    x_d = nc.dram_tensor("x", [C, H, W_], F32, kind="ExternalInput")
    wt_d = nc.dram_tensor("wt", [128, 8, 9, O], F16, kind="ExternalInput")
    bias_d = nc.dram_tensor("bias", [128, 1], F32, kind="ExternalInput")
    out_d = nc.dram_tensor("out", [O, H, W_], F32, kind="ExternalOutput")

    with TileContext(nc) as tc:
        with (
            tc.tile_pool(name="wpool", bufs=1) as wpool,
            tc.tile_pool(name="xpool", bufs=2) as xpool,
            tc.tile_pool(name="tpool", bufs=1) as tpool,
            tc.tile_pool(name="mpool", bufs=2) as mpool,
            tc.tile_pool(name="spool", bufs=3) as spool,
            tc.tile_pool(name="ppool", bufs=2, space="PSUM") as ppool,
        ):
            # --- ACT table warm-up: a 1-element activation issued before
            # anything else pulls in the Copy/Identity/Abs table set while
            # the first x DMAs are still in flight.
            scr = wpool.tile([1, 2], F32)
            nc.gpsimd.memset(scr[:, 0:1], 0.0)
            nc.scalar.activation(scr[:, 1:2], scr[:, 0:1], ACTF.Copy)

            wt = wpool.tile([128, 8, 9, O], F16)
            biasT = wpool.tile([128, 1], F32)
            maskT = wpool.tile([128, 1], U32)
            nc.gpsimd.memset(maskT[:], SIGN16)

            RG = R + 2

            def load_chunk(ci):
                """DMA chunk ci's x rows; top half on SP queue, bottom half
                on the Activation queue (parallel HW DMA-gen engines)."""
                r0 = ci * R
                xt = xpool.tile([128, R + 4, WP], F32, tag="xt")
                nc.gpsimd.memset(xt[:, :, 0:1], 0.0)
                nc.gpsimd.memset(xt[:, :, WP - 1:WP], 0.0)
                tlo, thi = r0 - 2, r0 + R + 2
                if tlo < 0:
                    nc.gpsimd.memset(xt[0:64, 0:-tlo, 1:WP - 1], 0.0)
                    nc.sync.dma_start(xt[0:64, -tlo:R + 4, 1:WP - 1],
                                      x_d[:, 0:thi, :])
                else:
                    nc.sync.dma_start(xt[0:64, :, 1:WP - 1], x_d[:, tlo:thi, :])
                blo, bhi = HH + r0 - 2, HH + r0 + R + 2
                if bhi > H:
                    nval = H - blo
                    nc.gpsimd.memset(xt[64:128, nval:R + 4, 1:WP - 1], 0.0)
                    nc.scalar.dma_start(xt[64:128, 0:nval, 1:WP - 1],
                                        x_d[:, blo:H, :])
                else:
                    nc.scalar.dma_start(xt[64:128, :, 1:WP - 1],
                                        x_d[:, blo:bhi, :])
                return xt

            def produce_monos(xt):
                """mono0 + Sobel gradients + sign-XOR monomials for one
                chunk.  Emitted one iteration AHEAD of the chunk's matmuls
                so every ACT cast precedes the previous chunk's PSUM evacs
                in the strict-FIFO ACT queue (otherwise the DVE XOR chain
                stalls on casts queued behind end-of-chunk evacs)."""
                monoE = mpool.tile([128, 4, RG, WP], F16, tag="monoE")
                nc.scalar.activation(monoE[:, 0], xt[:, 1:RG + 1, :], ACTF.Copy)

                # gy path first: its sign source unblocks mu4 earliest.
                # (All elementwise work stays on the DVE: offloading at/ut to
                # GPSIMD was measured 1.5x SLOWER overall - the shared SBUF
                # port inflates concurrent DVE ops from 2.8us to ~10us.)
                ut = tpool.tile([128, RG, WP], F32, tag="ut")
                nc.vector.tensor_sub(ut[:], xt[:, 0:RG, :], xt[:, 2:RG + 2, :])
                b2 = tpool.tile([128, RG, W_], F32, tag="b2")
                nc.vector.tensor_add(b2[:], ut[:, :, 0:WP - 2], ut[:, :, 2:WP])
                gy32 = tpool.tile([128, RG, WP], F32, tag="gy32")
                nc.gpsimd.memset(gy32[:, :, 0:1], 0.0)
                nc.gpsimd.memset(gy32[:, :, WP - 1:WP], 0.0)
                nc.vector.scalar_tensor_tensor(gy32[:, :, 1:WP - 1],
                                               ut[:, :, 1:WP - 1], 2.0, b2[:],
                                               ALU.mult, ALU.add)
                gyh = tpool.tile([128, RG, WP], F16, tag="gyh")
                nc.scalar.activation(gyh[:], gy32[:], ACTF.Copy)
                ay = tpool.tile([128, RG, W_], F32, tag="b2")  # b2 is dead
                nc.scalar.activation(ay[:], gy32[:, :, 1:WP - 1], ACTF.Abs)

                at = tpool.tile([128, RG, WP], F32, tag="at")
                nc.vector.tensor_add(at[:], xt[:, 0:RG, :], xt[:, 2:RG + 2, :])
                tt = tpool.tile([128, RG, WP], F32, tag="tt")
                nc.vector.scalar_tensor_tensor(tt[:], xt[:, 1:RG + 1, :], 2.0,
                                               at[:], ALU.mult, ALU.add)
                gx32 = tpool.tile([128, RG, WP], F32, tag="gx32")
                nc.gpsimd.memset(gx32[:, :, 0:1], 0.0)
                nc.gpsimd.memset(gx32[:, :, WP - 1:WP], 0.0)
                nc.vector.tensor_tensor(gx32[:, :, 1:WP - 1], tt[:, :, 0:WP - 2],
                                        tt[:, :, 2:WP], ALU.subtract)
                gxh = tpool.tile([128, RG, WP], F16, tag="gxh")
                nc.scalar.activation(gxh[:], gx32[:], ACTF.Copy)
                ax = tpool.tile([128, RG, W_], F32, tag="tt")  # tt is dead
                nc.scalar.activation(ax[:], gx32[:, :, 1:WP - 1], ACTF.Abs)

                # monomials y_S = x * chi_S (fp16), S=(sy<<2)|(sx<<1)|sd.
                # Early monomials (0,4,2,6) and late ones (1,5,3,7) live in
                # separate double-buffered tiles: the early tile's last PE
                # reader is the 4th m-group, so it frees at chunk MIDPOINT
                # and the next chunk's mono0 is never scheduler-gated on a
                # full previous chunk (that gating cost ~11.6us per chunk).
                mu = {S: monoE[:, i].bitcast(U32)
                      for i, S in enumerate((0, 4, 2, 6))}
                sy = gyh[:].bitcast(U32)
                sx = gxh[:].bitcast(U32)
                mk = maskT[:, 0:1]
                stt = nc.vector.scalar_tensor_tensor
                stt(mu[4], sy, mk, mu[0], ALU.bitwise_and, ALU.bitwise_xor)
                stt(mu[2], sx, mk, mu[0], ALU.bitwise_and, ALU.bitwise_xor)
                stt(mu[6], sx, mk, mu[4], ALU.bitwise_and, ALU.bitwise_xor)

                # e = |gy|-|gx| in fp32 (only its sign is used; fp16 anywhere
                # in the gradient path misbins too many pixels)
                e32 = tpool.tile([128, RG, WP], F32, tag="e32")
                nc.gpsimd.memset(e32[:, :, 0:1], 0.0)
                nc.gpsimd.memset(e32[:, :, WP - 1:WP], 0.0)
                nc.vector.tensor_tensor(e32[:, :, 1:WP - 1], ay[:], ax[:],
                                        ALU.subtract)
                eh = tpool.tile([128, RG, WP], F16, tag="gyh")  # gyh is dead
                nc.scalar.activation(eh[:], e32[:], ACTF.Copy)
                sd = eh[:].bitcast(U32)
                monoL = mpool.tile([128, 4, RG, WP], F16, tag="monoL")
                mu.update({S: monoL[:, i].bitcast(U32)
                           for i, S in enumerate((1, 5, 3, 7))})
                stt(mu[1], sd, mk, mu[0], ALU.bitwise_and, ALU.bitwise_xor)
                stt(mu[5], sd, mk, mu[4], ALU.bitwise_and, ALU.bitwise_xor)
                stt(mu[3], sd, mk, mu[2], ALU.bitwise_and, ALU.bitwise_xor)
                stt(mu[7], sd, mk, mu[6], ALU.bitwise_and, ALU.bitwise_xor)
                return {0: (monoE, 0), 4: (monoE, 1), 2: (monoE, 2),
                        6: (monoE, 3), 1: (monoL, 0), 5: (monoL, 1),
                        3: (monoL, 2), 7: (monoL, 3)}

            # chunk 0 prologue: load + full monomial production
            xt_cur = load_chunk(0)
            mono_cur = produce_monos(xt_cur)
            # weights go on the SP queue AFTER chunk 0's x rows: the x data
            # heads the critical path (DVE gradients), weights are only
            # needed once the first matmul issues.
            nc.sync.dma_start(wt[:], wt_d[:])
            nc.sync.dma_start(biasT[:], bias_d[:])

            pending_evac = None
            for ci in range(NCHUNK):
                r0 = ci * R
                msl = mono_cur

                # software pipeline: next chunk's x load + complete monomial
                # production are emitted (and run) during this chunk's
                # matmuls.
                if ci + 1 < NCHUNK:
                    xt_cur = load_chunk(ci + 1)
                    mono_cur = produce_monos(xt_cur)

                # ---- conv matmuls: m-MAJOR over both 4-row slots so each
                # monomial, once ready, feeds 2x the matmul work (better
                # runway during pipeline fill).  4 PSUM banks accumulate all
                # 72 (m,tap) contributions; 4-way PE tile packing per group.
                psum = {(sj, hb): ppool.tile([128, 512], F32, tag=f"ps{sj}{hb}",
                                             name=f"ps{sj}{hb}")
                        for sj in range(R // 4) for hb in range(2)}
                for mi, m in enumerate(MORDER):
                    mt, ms = msl[m]
                    for sj in range(R // 4):
                        ps_t = psum[(sj, 0)]
                        ps_b = psum[(sj, 1)]
                        for tap in range(9):
                            dy, dx = tap // 3, tap % 3
                            rA = 4 * sj + dy
                            rB = rA + 2
                            first = (mi == 0 and tap == 0)
                            st = (mi == 7 and tap == 8)
                            for (pr, ps, rr) in ((0, ps_t, rA), (64, ps_b, rA),
                                                 (0, ps_t, rB), (64, ps_b, rB)):
                                pc = 0 if rr == rA else 64
                                nc.tensor.matmul(
                                    ps[pc:pc + 64, :],
                                    wt[pr:pr + 64, m, tap, :],
                                    mt[pr:pr + 64, ms, rr:rr + 2, dx:dx + W_],
                                    start=first, stop=st,
                                    skip_group_check=True,
                                )
                # ---- evacuate PSUM (+bias) and store.  Evacs are emitted
                # one iteration LATE so their PE-completion waits are already
                # satisfied when the ACT queue reaches them - they can never
                # head-of-line-block the next chunk's casts.
                def evacuate(psum=psum, r0=r0):
                    for sj in range(R // 4):
                        y0 = r0 + 4 * sj
                        stg_t = spool.tile([128, 512], F32, tag="stg")
                        nc.scalar.activation(stg_t[:], psum[(sj, 0)][:],
                                             ACTF.Identity, bias=biasT[:, 0:1])
                        stg_b = spool.tile([128, 512], F32, tag="stg")
                        nc.scalar.activation(stg_b[:], psum[(sj, 1)][:],
                                             ACTF.Identity, bias=biasT[:, 0:1])
                        nc.sync.dma_start(out_d[:, y0:y0 + 2, :], stg_t[0:64])
                        nc.sync.dma_start(out_d[:, y0 + 2:y0 + 4, :],
                                          stg_t[64:128])
                        yb = HH + y0
                        nc.sync.dma_start(out_d[:, yb:yb + 2, :], stg_b[0:64])
                        nc.sync.dma_start(out_d[:, yb + 2:yb + 4, :],
                                          stg_b[64:128])
                if pending_evac is not None:
                    pending_evac()
                pending_evac = evacuate
            pending_evac()

    nc.compile()
    return nc


def _prep_host_inputs(Wfull: np.ndarray, bfull: np.ndarray):
    """Monomial weights wt[128, 8, 9, O] fp16 and bias[128,1] fp32."""
    sig = np.zeros((K, 3), np.float64)
    for k in range(K):
        a_, b_, c_ = (k >> 2) & 1, (k >> 1) & 1, k & 1
        Sy, Sx, D = a_, a_ ^ b_, b_ ^ c_
        sig[k] = [2 * Sy - 1, 2 * Sx - 1, 2 * D - 1]
    Wd = Wfull.astype(np.float64)  # (K, O, C, 3, 3)
    wt = np.zeros((64, 8, 9, O), np.float64)
    for S in range(8):
        coef = np.ones(K)
        if S & 4: coef = coef * sig[:, 0]
        if S & 2: coef = coef * sig[:, 1]
        if S & 1: coef = coef * sig[:, 2]
        Wp = np.einsum('k,kocyx->ocyx', coef, Wd) / 64.0  # (O, C, 3, 3)
        wt[:, S, :, :] = np.transpose(Wp.reshape(O, C, 9), (1, 2, 0))
    wt128 = np.concatenate([wt, wt], axis=0).astype(np.float16)
    bias = (bfull.astype(np.float64).sum(axis=0) / K).astype(np.float32)
    bias128 = np.concatenate([bias, bias])[:, None]
    return wt128, bias128


_NC_CACHE = None


def _get_nc():
    global _NC_CACHE
    if _NC_CACHE is None:
        _NC_CACHE = _build_nc()
    return _NC_CACHE


LAST_RESULT = None


def kernel(x: np.ndarray, W: np.ndarray, b: np.ndarray, **run_kwargs) -> np.ndarray:
    global LAST_RESULT
    assert x.shape == (B, C, H, W_) and W.shape == (K, O, C, 3, 3)
    nc = _get_nc()
    wt128, bias128 = _prep_host_inputs(np.asarray(W), np.asarray(b))
    xs = np.ascontiguousarray(np.asarray(x, dtype=np.float32))
    in_maps = [
        {"x": xs[i], "wt": wt128, "bias": bias128}
        for i in range(B)
    ]
    res = bass_utils.run_bass_kernel_spmd(nc, in_maps, core_ids=list(range(B)),
                                          **run_kwargs)
    LAST_RESULT = res
    out = np.stack([res.results[i]["out"] for i in range(B)], axis=0)
    return out.astype(np.float32)


if __name__ == "__main__":
    nc = _get_nc()
    print("built + compiled OK")
